# revision 52
# baseline (speedup 1.0000x reference)
"""GQA (B=2, L=2048, D=2048, H=16, KVH=4, HD=128) on 8 Trainium2 NeuronCores.

Sharding: core c = (batch b = c//4, kv-group g = c%4). Each core computes its
group's 4 query heads + 1 KV head end-to-end and a partial output projection
(Wo in-dim slice); the host sums the 4 partials per batch (tensor-parallel
unshard) -- no on-device collectives.

All four projections (Q/K/V/O) run as 3-term error-compensated fp8e4m3
DoubleRow matmuls at 3/4 the bf16 PE cost with bf16-level accuracy:
  W @ x ~= Whi@xhi + Wlo@xhi + Whi@xlo     (drop the Wlo@xlo term)
where Whi = e4m3(sW*W), Wlo = e4m3(sW*W - Whi) share one scale so all three
passes accumulate in one PSUM group (descale folded into evictions: rope
tables for Q/K, tensor_scalar for V, Copy-scale for O). Each DoubleRow
matmul packs two 128-k-tiles (K=256) at 0.5 cycles/row. Attention itself
(scores, softmax, P@V) stays bf16/fp16: e4m3 quantization of q/k/P/V was
measured at 2-3e-2 output error, over the 2e-2 gate.

The O projection's moving operand is the attention output quantized on the
fly: DVE writes ot_hi = e4m3(32*po/rowsum), gpsimd writes the residual
ot_lo; Wo DoubleRow-pairs two heads per matmul (K=256 over head-dim pairs).
The rowsum replicate matmul's `ones` tile carries 1/32 so the normalize
multiply lands pre-scaled for the e4m3 cast.

Software-pipelined single pass, same skeleton as the bf16 version: program
order interleaves projection chains (fillq) between attention blocks so the
in-order PE stream never drains while the Activation engine works through
the softmax exps. Output partials are written fp16 (host sums in fp32).

Other details unchanged from the bf16 version: transposed-score attention
(S.T = K_tile.T @ Q, one 128-j-tile per PSUM bank), max-free softmax with a
-30 diagonal mask bias via identity-stationary matmul, fp16 P tiles with
DVE-accumulated per-lane row sums, PE ones-matmul sum replicate, PSUM
budget 8 banks, flat-contiguous host-prearranged DMAs.
"""

import re
from contextlib import ExitStack

import ml_dtypes
import numpy as np

import concourse.bass as bass
import concourse.tile as tile
from concourse import mybir
from concourse.bass_utils import run_bass_kernel_spmd
from bass_rust import ScopedClock, VectorClock

dt = mybir.dt
BF16 = ml_dtypes.bfloat16
E4M3 = ml_dtypes.float8_e4m3
DR = mybir.MatmulPerfMode.DoubleRow

B, L, D = 2, 2048, 2048
H, KVH, HD = 16, 4, 128
G = H // KVH          # 4 query heads per kv head (= per core)
GD = G * HD           # 512: per-core q-head feature dim
THETA = 10000.0
SCALE = HD ** -0.5
NLT = L // 128        # 16 l-tiles
NDT = D // 128        # 16 d-tiles
NPR = NDT // 2        # 8 contraction k-tile pairs
NLC = L // 512        # 4 l-chunks

SW = 512.0            # weight quant scale (all of Wq/Wk/Wv/Wo)
SX = 16.0             # x quant scale
SOT = 32.0            # attention-out quant scale (folded into `ones`)
QK_DESCALE = 1.0 / (SW * SX)      # folded into rope tables
V_DESCALE = 1.0 / (SW * SX)       # V eviction tensor_scalar
O_DESCALE = 1.0 / (SW * SOT)      # output eviction scale


def _patch_tile_drain():
    """walrus in this container rejects multi-wait instructions on the SP
    queue; split the TileContext exit drain into one drain per proc."""
    def _drain_and_barrier_split(self, tick_clock, wait_clock):
        ticks = [int(s) for s in re.findall(r"\d+", str(tick_clock.global_clock))]
        for proc, t in enumerate(ticks):
            if t <= 0:
                continue
            vc = VectorClock()
            vc.require_at_least(proc, t)
            d = self.nc.sync.drain()
            wait_clock.add_sem_waits(d.ins, ScopedClock({None: vc}))
        self.nc.all_engine_barrier()
        assert self.sems is not None
        popped = self.nc._tile_sem_poison_stack.pop()
        assert popped is self._sem_poison
        self.nc.clear_and_free_semaphores(list(self.sems.allocated().values()))
        self.nc.all_engine_barrier()

    tile.TileContext._drain_and_barrier = _drain_and_barrier_split


def _split_multi_waits(nc):
    """This walrus build supports one sem-wait command per instruction; hoist
    excess waits onto same-engine NoOps inserted immediately before."""
    uid = 0
    for fn in nc.m.functions:
        for bb in fn.blocks:
            out = []
            for inst in bb.instructions:
                si = inst.sync_info
                if si is not None and si.on_wait and len(si.on_wait) > 1:
                    for w in si.on_wait[:-1]:
                        nop = mybir.InstNoOp(name=f"waitsplit-{uid}", ins=[], outs=[])
                        uid += 1
                        nop.engine = inst.engine
                        nop.sync_info = mybir.SyncInfo(on_wait=[w], on_update=[])
                        out.append(nop)
                    inst.sync_info = mybir.SyncInfo(
                        on_wait=[si.on_wait[-1]], on_update=si.on_update)
                out.append(inst)
            bb.instructions[:] = out


def _build_program():
    _patch_tile_drain()
    nc = bass.Bass("TRN2", target_bir_lowering=False, debug=False)

    # Host pre-arranges every tensor into its exact SBUF layout; all DMAs
    # are flat [128, N] contiguous copies. hi/lo pairs are e4m3 at a shared
    # scale so compensation passes accumulate in one PSUM group.
    xhid = nc.dram_tensor("xhid", [128, NLC * NDT * 512], dt.float8e4, kind="ExternalInput").ap()
    xlod = nc.dram_tensor("xlod", [128, NLC * NDT * 512], dt.float8e4, kind="ExternalInput").ap()
    wqhid = nc.dram_tensor("wqhid", [128, NDT * 512], dt.float8e4, kind="ExternalInput").ap()
    wqlod = nc.dram_tensor("wqlod", [128, NDT * 512], dt.float8e4, kind="ExternalInput").ap()
    wkvhid = nc.dram_tensor("wkvhid", [128, NDT * 256], dt.float8e4, kind="ExternalInput").ap()
    wkvlod = nc.dram_tensor("wkvlod", [128, NDT * 256], dt.float8e4, kind="ExternalInput").ap()
    wohid = nc.dram_tensor("wohid", [128, G * 2048], dt.float8e4, kind="ExternalInput").ap()
    wolod = nc.dram_tensor("wolod", [128, G * 2048], dt.float8e4, kind="ExternalInput").ap()
    ropeqd = nc.dram_tensor("ropeqd", [128, 2 * L], dt.bfloat16, kind="ExternalInput").ap()
    ropekd = nc.dram_tensor("ropekd", [128, 2 * L], dt.bfloat16, kind="ExternalInput").ap()
    trid = nc.dram_tensor("trid", [128, 256], dt.bfloat16, kind="ExternalInput").ap()
    outd = nc.dram_tensor("outd", [128, NDT, L], dt.float16, kind="ExternalOutput").ap()

    with tile.TileContext(nc) as tc:
        with ExitStack() as ctx:
            persist = ctx.enter_context(tc.tile_pool(name="persist", bufs=1))
            xpool = ctx.enter_context(tc.tile_pool(name="xchunk", bufs=2))
            ropep = ctx.enter_context(tc.tile_pool(name="rope", bufs=2))
            ptp = ctx.enter_context(tc.tile_pool(name="pt", bufs=17))
            smp = ctx.enter_context(tc.tile_pool(name="sm", bufs=2))
            stgp = ctx.enter_context(tc.tile_pool(name="stg", bufs=4))
            # PSUM: 8 banks total, [128,512] fp32 = 1 bank per tile
            psS = ctx.enter_context(tc.tile_pool(name="psS", bufs=3, space="PSUM"))
            psR = ctx.enter_context(tc.tile_pool(name="psR", bufs=1, space="PSUM"))
            psO = ctx.enter_context(tc.tile_pool(name="psO", bufs=2, space="PSUM"))
            psA = ctx.enter_context(tc.tile_pool(name="psA", bufs=2, space="PSUM"))

            # --- persistent SBUF residents ---
            wqhi = persist.tile([128, NDT * 512], dt.float8e4, tag="wqhi", name="wqhi")
            wqlo = persist.tile([128, NDT * 512], dt.float8e4, tag="wqlo", name="wqlo")
            wkvhi = persist.tile([128, NDT * 256], dt.float8e4, tag="wkvhi", name="wkvhi")
            wkvlo = persist.tile([128, NDT * 256], dt.float8e4, tag="wkvlo", name="wkvlo")
            wohi = persist.tile([128, G * 2048], dt.float8e4, tag="wohi", name="wohi")
            wolo = persist.tile([128, G * 2048], dt.float8e4, tag="wolo", name="wolo")
            ropeq_sb = persist.tile([128, 2 * L], dt.bfloat16, tag="ropeq", name="ropeq")
            ropek_sb = persist.tile([128, 2 * L], dt.bfloat16, tag="ropek", name="ropek")
            tri_sb = persist.tile([128, 256], dt.bfloat16, tag="tri", name="tri")
            ones_sb = persist.tile([128, 128], dt.float16, tag="ones", name="ones")
            # q/k/v/attention-out live in PER-CHUNK tiles: the Tile dep
            # tracker is tile-granular, so a single [HD, L] tile would give
            # every next-chunk write a false conflict with reads of already-
            # written chunks (measured as multi-us Ldweights stalls)
            qtc = [[persist.tile([HD, 512], dt.bfloat16, tag=f"qt{h}c{c}", name=f"qt{h}c{c}")
                    for c in range(NLC)] for h in range(G)]
            ktc = [persist.tile([HD, 512], dt.bfloat16, tag=f"ktc{c}", name=f"ktc{c}")
                   for c in range(NLC)]
            vc = [[persist.tile([128, HD], dt.float16, tag=f"vc{c}l{ls}", name=f"vc{c}l{ls}")
                   for ls in range(4)] for c in range(NLC)]
            # attention out, e4m3 hi/lo, per chunk: [p, h*512 + t]
            othi_c = [persist.tile([128, G * 512], dt.float8e4, tag=f"othi{c}", name=f"othi{c}")
                      for c in range(NLC)]
            otlo_c = [persist.tile([128, G * 512], dt.float8e4, tag=f"otlo{c}", name=f"otlo{c}")
                      for c in range(NLC)]

            wqhi3 = wqhi[:].rearrange("p (i n) -> p i n", i=NDT)
            wqlo3 = wqlo[:].rearrange("p (i n) -> p i n", i=NDT)
            wkvhi3 = wkvhi[:].rearrange("p (i n) -> p i n", i=NDT)
            wkvlo3 = wkvlo[:].rearrange("p (i n) -> p i n", i=NDT)
            wohi3 = wohi[:].rearrange("p (h n) -> p h n", h=G)
            wolo3 = wolo[:].rearrange("p (h n) -> p h n", h=G)
            othi3c = [t[:].rearrange("p (h n) -> p h n", h=G) for t in othi_c]
            otlo3c = [t[:].rearrange("p (h n) -> p h n", h=G) for t in otlo_c]

            # prologue DMAs (order matters: first Q matmuls need wq+x halves)
            xc_t = [None] * NLC          # (hi3, lo3) rearranged views

            def dma_x(lc):
                th = xpool.tile([128, NDT * 512], dt.float8e4, tag="xh", name="xh")
                tl = xpool.tile([128, NDT * 512], dt.float8e4, tag="xl", name="xl")
                nc.sync.dma_start(out=th, in_=xhid[:, lc * 8192:(lc + 1) * 8192])
                nc.sync.dma_start(out=tl, in_=xlod[:, lc * 8192:(lc + 1) * 8192])
                xc_t[lc] = (t3(th), t3(tl))

            def t3(t):
                return t[:].rearrange("p (i n) -> p i n", i=NDT)

            xc0h = xpool.tile([128, NDT * 512], dt.float8e4, tag="xh", name="xh")
            xc0l = xpool.tile([128, NDT * 512], dt.float8e4, tag="xl", name="xl")
            # HWDGE dispatch is 625ns per dma_start and serializes the whole
            # prologue, so use few, large transfers: a small head piece of
            # wq-hi/x-hi lets the A-pass start while the rest streams in.
            nc.sync.dma_start(out=wqhi[:, 0:2048], in_=wqhid[:, 0:2048])
            nc.sync.dma_start(out=xc0h[:, 0:2048], in_=xhid[:, 0:2048])
            nc.sync.dma_start(out=wkvhi, in_=wkvhid)
            nc.sync.dma_start(out=wqhi[:, 2048:8192], in_=wqhid[:, 2048:8192])
            nc.sync.dma_start(out=xc0h[:, 2048:8192], in_=xhid[:, 2048:8192])
            nc.sync.dma_start(out=wkvlo, in_=wkvlod)
            nc.sync.dma_start(out=wqlo, in_=wqlod)
            nc.sync.dma_start(out=xc0l, in_=xlod[:, 0:8192])
            # only the chunk-0 table columns are needed early
            nc.sync.dma_start(out=ropeq_sb[:, 0:512], in_=ropeqd[:, 0:512])
            nc.sync.dma_start(out=ropeq_sb[:, L:L + 512], in_=ropeqd[:, L:L + 512])
            nc.sync.dma_start(out=ropek_sb[:, 0:512], in_=ropekd[:, 0:512])
            nc.sync.dma_start(out=ropek_sb[:, L:L + 512], in_=ropekd[:, L:L + 512])
            xc_t[0] = (t3(xc0h), t3(xc0l))
            dma_x(1)
            nc.sync.dma_start(out=ropeq_sb[:, 512:L], in_=ropeqd[:, 512:L])
            nc.sync.dma_start(out=ropeq_sb[:, L + 512:2 * L], in_=ropeqd[:, L + 512:2 * L])
            nc.sync.dma_start(out=ropek_sb[:, 512:L], in_=ropekd[:, 512:L])
            nc.sync.dma_start(out=ropek_sb[:, L + 512:2 * L], in_=ropekd[:, L + 512:2 * L])
            nc.sync.dma_start(out=tri_sb, in_=trid)
            nc.sync.dma_start(out=wohi, in_=wohid)
            nc.sync.dma_start(out=wolo, in_=wolod)
            nc.gpsimd.memset(ones_sb, 1.0 / SOT)

            def rope_evict(ps, dst_slice, tables, lc, on_act=False):
                # tables [128, 2L]: cols [0:L) = cos*, [L:2L) = sin_eff*
                # (tables carry the 1/(SW*SX) descale, and SCALE for q)
                cs = tables[:, lc * 512:(lc + 1) * 512]
                sn = tables[:, L + lc * 512:L + (lc + 1) * 512]
                t1 = ropep.tile([128, 512], dt.bfloat16, tag="t1", name="t1")
                t2 = ropep.tile([128, 512], dt.bfloat16, tag="t2", name="t2")
                if on_act:
                    # chunk 0: Activation is idle before attention starts, so
                    # evict PSUM through it and keep the DVE ops tiny (bf16)
                    raw = ropep.tile([128, 512], dt.bfloat16, tag="raw", name="raw")
                    swp = ropep.tile([128, 512], dt.bfloat16, tag="swp", name="swp")
                    nc.scalar.copy(raw, ps)
                    nc.scalar.copy(swp[0:64, :], ps[64:128, :])
                    nc.scalar.copy(swp[64:128, :], ps[0:64, :])
                    nc.vector.tensor_tensor(t2, raw, cs, mybir.AluOpType.mult)
                    nc.vector.tensor_tensor(t1, swp, sn, mybir.AluOpType.mult)
                else:
                    nc.vector.tensor_tensor(t2, ps, cs, mybir.AluOpType.mult)
                    nc.vector.tensor_tensor(t1[0:64, :], ps[64:128, :], sn[0:64, :],
                                            mybir.AluOpType.mult)
                    nc.vector.tensor_tensor(t1[64:128, :], ps[0:64, :], sn[64:128, :],
                                            mybir.AluOpType.mult)
                nc.gpsimd.tensor_tensor(dst_slice, t1, t2, mybir.AluOpType.add)

            class FillQueue:
                """PE filler micro-ops (cost_ns, emit_fn) consumed between
                attention blocks to cover the Activation engine's per-block
                fixed latency."""
                def __init__(self):
                    self.items = []

                def add(self, cost, fn):
                    self.items.append((cost, fn))

                def consume(self, budget):
                    while budget > 0 and self.items:
                        c, fn = self.items.pop(0)
                        fn()
                        budget -= c

                def drain(self):
                    self.consume(float("inf"))

            fillq = FillQueue()

            def a_q(lc, ot, queue=None):
                # Q runs 2-term ('w' mode: Whi@xhi + Wlo@xhi, no x-residual
                # pass): Q is the largest projection and the scores path
                # tolerates the x-quantization error (sim: 1.25e-2 < 2e-2)
                xh3, xl3 = xc_t[lc]
                state = {}

                def stepAB(p):
                    if p == 0:
                        state["ps"] = psA.tile([128, 512], dt.float32, tag="psA", name="psA")
                    wslice = (slice(None), slice(2 * p, 2 * p + 2),
                              slice(ot * 128, (ot + 1) * 128))
                    nc.tensor.matmul(state["ps"], wqhi3[wslice], xh3[:, 2 * p:2 * p + 2, :],
                                     start=(p == 0), stop=False, perf_mode=DR)
                    nc.tensor.matmul(state["ps"], wqlo3[wslice], xh3[:, 2 * p:2 * p + 2, :],
                                     start=False, stop=(p == NPR - 1), perf_mode=DR)
                    if p == NPR - 1:
                        rope_evict(state["ps"], qtc[ot][lc][:, 0:512], ropeq_sb, lc)
                for p in range(NPR):
                    if queue is None:
                        stepAB(p)
                    else:
                        queue.add(213, (lambda p=p: stepAB(p)))

            def a_k(lc, queue=None):
                xh3, xl3 = xc_t[lc]
                state = {}

                def stepAB(p):
                    if p == 0:
                        state["ps"] = psA.tile([128, 512], dt.float32, tag="psA", name="psA")
                    wslice = (slice(None), slice(2 * p, 2 * p + 2), slice(0, 128))
                    nc.tensor.matmul(state["ps"], wkvhi3[wslice], xh3[:, 2 * p:2 * p + 2, :],
                                     start=(p == 0), stop=False, perf_mode=DR)
                    nc.tensor.matmul(state["ps"], wkvlo3[wslice], xh3[:, 2 * p:2 * p + 2, :],
                                     start=False, stop=False, perf_mode=DR)

                def stepC(p):
                    wslice = (slice(None), slice(2 * p, 2 * p + 2), slice(0, 128))
                    nc.tensor.matmul(state["ps"], wkvhi3[wslice], xl3[:, 2 * p:2 * p + 2, :],
                                     start=False, stop=(p == NPR - 1), perf_mode=DR)
                    if p == NPR - 1:
                        rope_evict(state["ps"], ktc[lc][:, 0:512], ropek_sb, lc)
                for p in range(NPR):
                    if queue is None:
                        stepAB(p)
                    else:
                        queue.add(213, (lambda p=p: stepAB(p)))
                for p in range(NPR):
                    if queue is None:
                        stepC(p)
                    else:
                        queue.add(107, (lambda p=p: stepC(p)))

            def a_v_chain(lc, ls):
                    xh3, xl3 = xc_t[lc]
                    # psO is idle between attention groups; keeping V off psS
                    # protects the cross-group warm score tiles from reuse
                    pv = psO.tile([128, 512], dt.float32, tag="psO", name="pv")
                    xsl = (slice(None), None, slice(ls * 128, (ls + 1) * 128))
                    for p in range(NPR):
                        xp = (slice(None), slice(2 * p, 2 * p + 2), xsl[2])
                        vh = (slice(None), slice(2 * p, 2 * p + 2), slice(128, 256))
                        nc.tensor.matmul(pv[:, 0:128], xh3[xp], wkvhi3[vh],
                                         start=(p == 0), stop=False, perf_mode=DR)
                        nc.tensor.matmul(pv[:, 0:128], xl3[xp], wkvhi3[vh],
                                         start=False, stop=False, perf_mode=DR)
                    for p in range(NPR):
                        xp = (slice(None), slice(2 * p, 2 * p + 2), xsl[2])
                        vl = (slice(None), slice(2 * p, 2 * p + 2), slice(128, 256))
                        nc.tensor.matmul(pv[:, 0:128], xh3[xp], wkvlo3[vl],
                                         start=False, stop=(p == NPR - 1), perf_mode=DR)
                    nc.vector.tensor_scalar(
                        vc[lc][ls][:, 0:128],
                        pv[:, 0:128], V_DESCALE, None, mybir.AluOpType.mult)

            def a_v(lc, queue=None):
                for ls in range(4):
                    if queue is None:
                        a_v_chain(lc, ls)
                    else:
                        queue.add(640, (lambda ls=ls: a_v_chain(lc, ls)))

            def emit_scores(c, h, j):
                r = j - 4 * c
                off = r * 128 if r >= 0 else 0
                S = psS.tile([128, 512], dt.float32, tag="psS", name="psS")
                diag = r >= 0
                nc.tensor.matmul(S[:, off:], ktc[j // 4][:, (j % 4) * 128:(j % 4 + 1) * 128],
                                 qtc[h][c][:, off:512],
                                 start=True, stop=not diag)
                if diag:
                    # causal mask: accumulate a -30 bias onto the masked
                    # entries of the diagonal block (identity-stationary
                    # matmul adds the bias tile); exp then flushes them
                    # to exact fp16 zeros
                    nc.tensor.matmul(S[:, off:off + 128], tri_sb[:, 128:256],
                                     tri_sb[:, 0:128], start=False, stop=True)
                return S, off

            def b_head(c, h, pending=None, warm=None, nxt=None, budget=300,
                       lo_on_dve=False):
                """Emits one head's attention blocks with a two-block score
                pipeline: the next blocks' score matmuls are emitted before
                this block's PV so the in-order PE has independent work
                covering the exp + semaphore latency. `warm` carries score
                tiles pre-emitted during the previous head; `nxt` names the
                following head so this head's last iterations pre-emit its
                first scores. Returns (tail_closure, warm_for_next)."""
                njt = 4 * (c + 1)
                po = psO.tile([128, 512], dt.float32, tag="psO", name="psO")
                acc = smp.tile([128, 512], dt.float16, tag="acc", name="acc")
                depth = 3
                pipe = list(warm) if warm else [emit_scores(c, h, j) for j in range(depth)]
                warm_out = []
                for j in range(njt):
                    S, off = pipe.pop(0)
                    if j + depth < njt:
                        pipe.append(emit_scores(c, h, j + depth))
                    elif nxt is not None:
                        warm_out.append(emit_scores(nxt[0], nxt[1], j + depth - njt))
                    pt = ptp.tile([128, 512], dt.float16, tag="pt", name="pt")
                    nc.scalar.activation(pt[:, off:], S[:, off:],
                                         mybir.ActivationFunctionType.Exp)
                    nc.tensor.matmul(po[:, off:],
                                     vc[j // 4][j % 4][:, 0:128],
                                     pt[:, off:],
                                     start=(j == 0), stop=(j == njt - 1))
                    if j == 0:
                        nc.vector.tensor_copy(acc, pt)
                    else:
                        nc.vector.tensor_tensor(acc[:, off:], acc[:, off:], pt[:, off:],
                                                mybir.AluOpType.add)
                    if j == 1 and pending is not None:
                        pending()
                        pending = None
                    fillq.consume(budget)
                if pending is not None:
                    pending()

                def tail():
                    # replicate per-lane partial sums across partitions on the
                    # PE (ones carries 1/SOT so the normalized product lands
                    # pre-scaled for the e4m3 cast), reciprocal + normalize,
                    # then quantize: DVE writes ot_hi, gpsimd the residual.
                    fillq.consume(350)
                    R = psR.tile([128, 512], dt.float32, tag="psR", name="psR")
                    nc.tensor.matmul(R, ones_sb, acc, start=True, stop=True)
                    rcp = smp.tile([128, 512], dt.float32, tag="rcp", name="rcp")
                    nc.vector.reciprocal(rcp, R)
                    t32 = smp.tile([128, 512], dt.float32, tag="t32", name="t32")
                    nc.vector.tensor_tensor(t32, po, rcp, mybir.AluOpType.mult)
                    hi_sl = othi_c[c][:, h * 512:(h + 1) * 512]
                    lo_sl = otlo_c[c][:, h * 512:(h + 1) * 512]
                    nc.vector.tensor_copy(hi_sl, t32)
                    if lo_on_dve:
                        # final heads: the residual write is on the kernel's
                        # critical tail, and DVE (533ns) beats the Pool queue
                        nc.vector.tensor_tensor(lo_sl, t32, hi_sl,
                                                mybir.AluOpType.subtract)
                    else:
                        nc.gpsimd.tensor_tensor(lo_sl, t32, hi_sl,
                                                mybir.AluOpType.subtract)
                return tail, warm_out

            def c_quarter(lc, quarter, split_dma=False, queue=None, stagger=False):
                state = {}

                def w_mm(pw, et, hp, pss, first=False, last=False):
                    # one compensation pass for head-pair hp: pss selects
                    # (w_hi, ot_hi) / (w_lo, ot_hi) / (w_hi, ot_lo)
                    w3 = wohi3 if pss in (0, 2) else wolo3
                    o3 = othi3c[lc] if pss in (0, 1) else otlo3c[lc]
                    nc.tensor.matmul(
                        pw,
                        w3[:, 2 * hp:2 * hp + 2, et * 128:(et + 1) * 128],
                        o3[:, 2 * hp:2 * hp + 2, :],
                        start=first, stop=last, perf_mode=DR)

                def full_chain(pw, et):
                    for hp in range(2):
                        for pss in range(3):
                            w_mm(pw, et, hp, pss, first=(hp == 0 and pss == 0),
                                 last=(hp == 1 and pss == 2))

                def evict(pw, stg, k, et):
                    dst = stg[:, k * 512:(k + 1) * 512]
                    if split_dma and quarter == 3 and k == 3:
                        # split the final eviction+DMA unevenly so the very
                        # last piece through the drain tail is small
                        nc.vector.tensor_scalar(dst[:, 0:384], pw[:, 0:384],
                                                O_DESCALE, None, mybir.AluOpType.mult)
                        nc.sync.dma_start(out=outd[:, et:et + 1, lc * 512:lc * 512 + 384],
                                          in_=dst[:, 0:384])
                        nc.scalar.activation(dst[:, 384:512], pw[:, 384:512],
                                             mybir.ActivationFunctionType.Copy,
                                             scale=O_DESCALE)
                        nc.sync.dma_start(out=outd[:, et:et + 1, lc * 512 + 384:(lc + 1) * 512],
                                          in_=dst[:, 384:512])
                        return
                    evict_dve = (k % 2 == 0) if split_dma else (k % 4 != 1)
                    if stagger:
                        evict_dve = (k % 2 == 1)
                    if evict_dve:
                        nc.vector.tensor_scalar(dst, pw, O_DESCALE, None,
                                                mybir.AluOpType.mult)
                    else:
                        nc.scalar.activation(dst, pw, mybir.ActivationFunctionType.Copy,
                                             scale=O_DESCALE)
                    if split_dma and quarter < 3:
                        # pair-batched writeback: halves the HWDGE dispatch
                        # count (625ns each) that otherwise stretches the
                        # drain tail, while still starting mid-quarter
                        if k % 2 == 1:
                            e0 = quarter * 4 + k - 1
                            nc.sync.dma_start(
                                out=outd[:, e0:e0 + 2, lc * 512:(lc + 1) * 512],
                                in_=stg[:, (k - 1) * 512:(k + 1) * 512])
                    elif split_dma:
                        nc.sync.dma_start(
                            out=outd[:, et:et + 1, lc * 512:(lc + 1) * 512],
                            in_=dst)
                    elif k == 3:
                        nc.sync.dma_start(
                            out=outd[:, quarter * 4:(quarter + 1) * 4, lc * 512:(lc + 1) * 512],
                            in_=stg)

                def chain(k):
                    if k == 0:
                        state["stg"] = stgp.tile([128, 2048], dt.float16, tag="stg", name="stg")
                    stg = state["stg"]
                    et = quarter * 4 + k
                    if stagger and k == 0:
                        # all four chains emit their head-pair-0 passes (which
                        # depend only on heads 0/1) before any chain's
                        # head-pair-1 passes (which wait on the final head's
                        # normalize+quantize) so the in-order PE queue has
                        # ready work while that tail drains
                        state["pw0"] = psA.tile([128, 512], dt.float32, tag="psA", name="psA")
                        state["pw1"] = psA.tile([128, 512], dt.float32, tag="psA", name="psA")
                        state["pw2"] = psS.tile([128, 512], dt.float32, tag="psS", name="psS")
                        state["pw3"] = psS.tile([128, 512], dt.float32, tag="psS", name="psS")
                        for k2 in range(4):
                            for pss in range(3):
                                w_mm(state[f"pw{k2}"], et + k2, 0, pss, first=(pss == 0))
                        for pss in range(3):
                            w_mm(state["pw0"], et, 1, pss, last=(pss == 2))
                        evict(state["pw0"], stg, k, et)
                        return
                    if stagger and k in (1, 2, 3):
                        for pss in range(3):
                            w_mm(state[f"pw{k}"], et, 1, pss, last=(pss == 2))
                        evict(state[f"pw{k}"], stg, k, et)
                        return
                    if split_dma:
                        # post-attention: psS/psO/psR are free, so rotate the
                        # chain banks across all pools -- two psA banks alone
                        # recycle slower (evict + sem) than a 0.64us chain
                        pool, tag = (psA, "psA") if k % 2 == 0 else (psS, "psS")
                    else:
                        pool, tag = psA, "psA"
                    pw = pool.tile([128, 512], dt.float32, tag=tag, name=tag)
                    full_chain(pw, et)
                    evict(pw, stg, k, et)
                for k in range(4):
                    if queue is None:
                        chain(k)
                    else:
                        queue.add(640, (lambda k=k: chain(k)))

            def a_chunk0_interleaved():
                """Chunk 0 is DMA-paced: run all five projection chains
                pair-by-pair so the PE tracks x/wq piece arrivals instead of
                stalling a full chain on the last piece. Passes A+B run
                pair-major first (they need only the hi pieces + wlo), then
                pass C (needs xlo, which lands after)."""
                xh3, xl3 = xc_t[0]
                # chain = (psum_tile, whi_fn, wlo_fn, evict_fn, has_pass_c)
                chains = []
                for ot in range(G):
                    tag = "psA" if ot < 2 else "psS"
                    pool = psA if ot < 2 else psS
                    ps = pool.tile([128, 512], dt.float32, tag=tag, name="psc0")
                    sl = (lambda p, ot=ot: (slice(None), slice(2 * p, 2 * p + 2),
                                            slice(ot * 128, (ot + 1) * 128)))
                    chains.append((ps,
                                   (lambda p, sl=sl: wqhi3[sl(p)]),
                                   (lambda p, sl=sl: wqlo3[sl(p)]),
                                   (lambda ps=ps, ot=ot: rope_evict(
                                       ps, qtc[ot][0][:, 0:512], ropeq_sb, 0,
                                       on_act=True)),
                                   False))     # Q is 2-term: no x-residual pass
                ps = psS.tile([128, 512], dt.float32, tag="psS", name="psc0")
                ksl = lambda p: (slice(None), slice(2 * p, 2 * p + 2), slice(0, 128))
                chains.append((ps,
                               (lambda p: wkvhi3[ksl(p)]),
                               (lambda p: wkvlo3[ksl(p)]),
                               (lambda ps=ps: rope_evict(ps, ktc[0][:, 0:512], ropek_sb, 0)),
                               True))
                # pass-major: the A pass needs only the hi tensors (first
                # dispatches); pair-outer within a pass so a chain stalled on
                # the next DMA piece never blocks the others. K's remaining
                # passes run before Q's so ktc0 (which gates the first score
                # matmul of the whole kernel) evicts as early as possible --
                # on the DVE path, since the on_act path would also work but
                # K finishing last used to push it behind the exp stream.
                for p in range(NPR):
                    for ps_, whi_fn, wlo_fn, evict_fn, has_c in chains:
                        nc.tensor.matmul(ps_, whi_fn(p), xh3[:, 2 * p:2 * p + 2, :],
                                         start=(p == 0), stop=False, perf_mode=DR)
                ps_, whi_fn, wlo_fn, evict_fn, _ = chains[-1]      # K chain
                for p in range(NPR):
                    nc.tensor.matmul(ps_, wlo_fn(p), xh3[:, 2 * p:2 * p + 2, :],
                                     start=False, stop=False, perf_mode=DR)
                for p in range(NPR):
                    nc.tensor.matmul(ps_, whi_fn(p), xl3[:, 2 * p:2 * p + 2, :],
                                     start=False, stop=(p == NPR - 1), perf_mode=DR)
                    if p == NPR - 1:
                        evict_fn()
                for p in range(NPR):
                    for ps_, whi_fn, wlo_fn, evict_fn, has_c in chains[:-1]:
                        nc.tensor.matmul(ps_, wlo_fn(p), xh3[:, 2 * p:2 * p + 2, :],
                                         start=False, stop=(p == NPR - 1),
                                         perf_mode=DR)
                        if p == NPR - 1:
                            evict_fn()

            # software-pipelined emission: projection/output-projection PE
            # micro-ops are streamed between attention blocks (fillq) so the
            # PE stream never drains while Activation works through the exps
            a_chunk0_interleaved()
            a_v(0)
            dma_x(2)
            a_q(1, 0, fillq)
            a_k(1, fillq)
            a_q(1, 1, fillq)
            a_v(1, fillq)
            a_q(1, 2, fillq)
            a_q(1, 3, fillq)
            warm = None
            for h in range(G):
                nxt = (0, h + 1) if h + 1 < G else (1, 0)
                tail, warm = b_head(0, h, warm=warm, nxt=nxt)
                tail()
            fillq.drain()
            dma_x(3)
            a_q(2, 0)
            a_k(2, fillq)
            a_v(2, fillq)
            a_q(2, 1, fillq)
            a_q(2, 2, fillq)
            a_q(2, 3, fillq)
            for h in range(G):
                nxt = (1, h + 1) if h + 1 < G else (2, 0)
                tail, warm = b_head(1, h, warm=warm, nxt=nxt)
                tail()
            fillq.drain()
            a_q(3, 0)
            a_k(3, fillq)
            a_v(3, fillq)
            a_q(3, 1, fillq)
            a_q(3, 2, fillq)
            a_q(3, 3, fillq)
            for h in range(G):
                nxt = (2, h + 1) if h + 1 < G else (3, 0)
                tail, warm = b_head(2, h, warm=warm, nxt=nxt)
                tail()
            fillq.drain()
            for q in range(4):
                c_quarter(0, q, queue=fillq)
            for q in range(4):
                c_quarter(1, q, queue=fillq)
            tail, warm = b_head(3, 0, warm=warm, nxt=(3, 1))
            tail()
            tail, warm = b_head(3, 1, warm=warm, nxt=(3, 2))
            tail()
            tail, warm = b_head(3, 2, warm=warm, nxt=(3, 3))
            tail()
            for q in range(4):
                c_quarter(2, q, queue=fillq)
            tail, _ = b_head(3, 3, warm=warm)
            tail()
            fillq.drain()
            for q in range(4):
                c_quarter(3, q, split_dma=True, stagger=(q == 0))

    _split_multi_waits(nc)
    return nc


_PROG = None


def _rope_tables():
    inv_freq = 1.0 / (THETA ** (np.arange(0, HD, 2, dtype=np.float32) / HD))
    t = np.arange(L, dtype=np.float32)
    freqs = np.outer(t, inv_freq)
    emb = np.concatenate([freqs, freqs], axis=-1)      # [L, HD]
    cos = np.cos(emb).T.copy()                         # [HD, L]
    sin = np.sin(emb).T.copy()
    sin_eff = sin.copy()
    sin_eff[:64] = -sin_eff[:64]                       # dest-indexed rotate_half sign
    return cos, sin_eff


def _hilo(a, scale):
    """e4m3 hi/lo at a shared scale (all compensation passes accumulate in
    one PSUM group; the descale is folded into the eviction)."""
    s = np.ascontiguousarray(a).astype(np.float32) * scale
    hi = s.astype(E4M3)
    lo = (s - hi.astype(np.float32)).astype(E4M3)
    return hi, lo


def _prepare_in_maps(x, Wq, Wk, Wv, Wo):
    cos, sin_eff = _rope_tables()
    bfc = lambda a: np.ascontiguousarray(a).astype(BF16)
    ropeq = bfc(np.concatenate([cos * (SCALE * QK_DESCALE),
                                sin_eff * (SCALE * QK_DESCALE)], axis=1))  # [128, 2L]
    ropek = bfc(np.concatenate([cos * QK_DESCALE, sin_eff * QK_DESCALE], axis=1))
    # cols 0:128 = -30 on masked entries (pj > fq), cols 128:256 = identity
    negtri = -30.0 * (1.0 - np.tril(np.ones((128, 128), dtype=np.float32)).T)
    tri = bfc(np.concatenate([negtri, np.eye(128, dtype=np.float32)], axis=1))

    x, Wq, Wk, Wv, Wo = (np.asarray(a, dtype=np.float32) for a in (x, Wq, Wk, Wv, Wo))
    # xd[p, lc*8192 + i*512 + t] = x[b][lc*512+t, i*128+p]
    xhilo = []
    for b in range(B):
        xT = x[b].T                                   # [D, L]
        lay = (xT.reshape(NDT, 128, NLC, 512).transpose(1, 2, 0, 3)
               .reshape(128, NLC * NDT * 512))
        xhilo.append(_hilo(lay, SX))
    in_maps = []
    for c in range(8):
        b, g = c // 4, c % 4
        wqT = Wq[g * GD:(g + 1) * GD, :].T            # [D, GD]
        wqd = wqT.reshape(NDT, 128, GD).transpose(1, 0, 2).reshape(128, NDT * GD)
        wkT = Wk[g * HD:(g + 1) * HD, :].T            # [D, HD]
        wvT = Wv[g * HD:(g + 1) * HD, :].T
        wkv = np.concatenate(
            [wkT.reshape(NDT, 128, HD), wvT.reshape(NDT, 128, HD)], axis=2)
        wkvd = wkv.transpose(1, 0, 2).reshape(128, NDT * 256)
        woT = Wo[:, g * GD:(g + 1) * GD].T            # [GD, D]
        wod = woT.reshape(G, 128, D).transpose(1, 0, 2).reshape(128, G * D)
        wqhi, wqlo = _hilo(wqd, SW)
        wkvhi, wkvlo = _hilo(wkvd, SW)
        wohi, wolo = _hilo(wod, SW)
        in_maps.append({
            "xhid": xhilo[b][0], "xlod": xhilo[b][1],
            "wqhid": wqhi, "wqlod": wqlo,
            "wkvhid": wkvhi, "wkvlod": wkvlo,
            "wohid": wohi, "wolod": wolo,
            "ropeqd": ropeq, "ropekd": ropek,
            "trid": tri,
        })
    return in_maps


def _run(in_maps, **kwargs):
    global _PROG
    if _PROG is None:
        _PROG = _build_program()
    return run_bass_kernel_spmd(_PROG, in_maps, list(range(8)), **kwargs)


def _gather(res):
    out = np.zeros((B, L, D), dtype=np.float32)
    for c in range(8):
        b = c // 4
        outd = res.results[c]["outd"].astype(np.float32)   # [128, 16, 2048]
        part = outd.transpose(1, 0, 2).reshape(D, L)       # [e, seq]
        out[b] += part.T
    return out


def kernel(x, Wq, Wk, Wv, Wo):
    return _gather(_run(_prepare_in_maps(x, Wq, Wk, Wv, Wo)))
